# revision 39
# baseline (speedup 1.0000x reference)
"""Multi-head attention (B=2, S=2048, H=1024, 16 heads) on 8 TRN2 NeuronCores.

Sharding: data parallel on batch (2) x tensor parallel on heads (4 heads/core,
Megatron column-split qkv, row-split wo). Host pre-transposes and pre-TILES
x/y/weights into the exact SBUF layouts (so every input DMA is a handful of
fully-contiguous 4-8KB-per-partition transfers; the HWDGE rings are
descriptor-rate-bound), pre-scales wq by dh^-0.5, and sum-reduces the 4
partial outputs per batch element.

Per-core kernel (v3):
  Steady state is tensor-issue-bound at ~1.19us/step (QK pair 0.31 + PV 0.43
  + woven projections 0.44); the exp chain floor is ~1.0us/step (back-to-back
  ACTIVATEs overlap their pipeline fill/drain). The first ~16 steps are
  DMA-arrival + V/K-projection-crunch bound: both HWDGE rings lead with y
  (K-proj and V-proj both consume it), j0 split in halves so the first
  projection group starts on the first half-MB; x-j123/wo ride last. Weave
  deadlines are arrival-aware so a not-yet-ready group never head-of-line
  blocks ready work, and Q-projections for later q-blocks are pulled into
  the post-crunch trough so late steps run ACT-bound. Warmup matmuls ramp
  the PE clock governor (~6us of sustained activity to reach peak; it
  drops back during idle gaps).

  Attention per 512-wide q-block and head pair: row-tiled (2-head packed)
  QK^T -> logitsT psum [128,1024] -> one ACT exp per pair (psum->sbuf bf16)
  -> PV matmul with fused denominator row (fp32 accumulate) -> fast
  reciprocal + gpsimd partition_broadcast normalize -> pair-stacked bf16
  output projection, deferred one block for overlap. The final pair instead
  normalizes via a K=1 broadcast matmul (tensor is idle at the tail) and the
  final q-block's output projection alternates psum pools and vector/scalar
  evictions with immediate per-chunk DMA. Output stays f32 (bf16 DRAM
  outputs do not land on HW). fp8 was tried and rejected: e4m3 DoubleRow
  halves instructions but NOT stream time (no throughput win), and q/k-proj
  quantization alone costs 3.2e-2 rel err vs the 2e-2 gate.
"""
import sys
sys.path.insert(0, '/opt/trn_rl_repo')
from contextlib import ExitStack

import numpy as np
import ml_dtypes

import concourse.bacc as bacc
import concourse.tile as tile
from concourse import mybir
from concourse import bass_utils

B, S, H, NH = 2, 2048, 1024, 16
DH = H // NH            # 64
NCORES = 8
HPC = NH // (NCORES // B)   # 4 heads per core
C = HPC * DH            # 256 projected cols per core
KT_H = H // 128         # 8 contraction tiles over H
SK = S // 128           # 16 s-subtiles
JBLK = 512
NJ = S // JBLK          # 4 q-blocks
F32 = mybir.dt.float32
BF16 = mybir.dt.bfloat16

_CACHE = {}
_DEBUG = False


def _build():
    nc = bacc.Bacc('TRN2', debug=False, num_devices=NCORES)
    # All activations/weights arrive HOST-PRE-TILED into the SBUF layout so
    # every chunk DMA is fully contiguous per partition (8KB lines): the
    # HWDGE rings are descriptor-rate-bound, so line size sets arrival BW.
    xj = nc.dram_tensor('xj', [NJ, 128, KT_H, JBLK], BF16, kind='ExternalInput')
    yj = nc.dram_tensor('yj', [NJ, 128, KT_H, JBLK], BF16, kind='ExternalInput')
    wq = nc.dram_tensor('wq', [128, KT_H, C], BF16, kind='ExternalInput')
    wk = nc.dram_tensor('wk', [128, KT_H, C], BF16, kind='ExternalInput')
    wv = nc.dram_tensor('wv', [128, KT_H, C], BF16, kind='ExternalInput')
    wo = nc.dram_tensor('wo', [128, 2, H], BF16, kind='ExternalInput')
    ebias = nc.dram_tensor('ebias', [128, SK], F32, kind='ExternalInput')
    out = nc.dram_tensor('out', [S, H], F32, kind='ExternalOutput')
    dbg = {}
    if _DEBUG:
        for nm, w in [('d_kt', S), ('d_qt', S), ('d_ex', 2 * JBLK),
                      ('d_v', HPC * (DH + 1)), ('d_wo', 2 * H),
                      ('d_ctx', JBLK), ('d_raw', JBLK)]:
            dbg[nm] = nc.dram_tensor(nm, [128, w], F32, kind='ExternalOutput')

    with tile.TileContext(nc) as tc, ExitStack() as ctx:
        res = ctx.enter_context(tc.tile_pool(name='res', bufs=1))
        expool = ctx.enter_context(tc.tile_pool(name='expool', bufs=4))
        ctxpool = ctx.enter_context(tc.tile_pool(name='ctxpool', bufs=2))
        small = ctx.enter_context(tc.tile_pool(name='small', bufs=3))
        outpool = ctx.enter_context(tc.tile_pool(name='outpool', bufs=4))
        ps_qk = ctx.enter_context(tc.tile_pool(name='ps_qk', bufs=2, space='PSUM'))
        ps_acc = ctx.enter_context(tc.tile_pool(name='ps_acc', bufs=2, space='PSUM'))
        ps_g = ctx.enter_context(tc.tile_pool(name='ps_g', bufs=2, space='PSUM'))

        # ---- input DMAs: few, large, contiguous, split across both rings ----
        # HWDGE rings execute FIFO per issuing engine, so arrival order ==
        # issue order per ring; the two rings share the ~358GB/s HBM pipe.
        # K/V-path (wk, y, wv) rides sync; Q-path (wq, x) rides scalar.
        eb = res.tile([128, SK], F32, tag='eb')
        wk_r = res.tile([128, KT_H, C], BF16, tag='wk')
        wv_r = res.tile([128, KT_H, C], BF16, tag='wv')
        wq_r = res.tile([128, KT_H, C], BF16, tag='wq')
        wo_r = res.tile([128, 2, H], BF16, tag='wo')
        xr = res.tile([128, NJ, KT_H, JBLK], BF16, tag='xr')
        yr = res.tile([128, NJ, KT_H, JBLK], BF16, tag='yr')
        xts = [[xr[:, j, k] for j in range(NJ)] for k in range(KT_H)]
        yts = [[yr[:, j, k] for j in range(NJ)] for k in range(KT_H)]

        # Each dma_start costs ~2us of serial ring time on top of transfer
        # (completion receipt), so the critical path uses FEW, LARGE DMAs
        # spread over THREE rings (sync + scalar HWDGE, gpsimd SWDGE). y
        # gates the whole first q-block (K-proj and V-proj both consume
        # it); x-j123/wo aren't touched until step ~26+ and ride last.
        # Ring constraints: a ring's queue is FIFO, and an engine's OTHER
        # work queues behind its DMA transfers. So the scalar ring carries
        # only tiny DMAs that finish before the first exp; the gpsimd ring
        # only what finishes before its first in-loop op (V-denominator
        # muls); everything else rides sync in consumption order.
        nc.sync.dma_start(out=wk_r, in_=wk.ap())
        nc.sync.dma_start(out=yr[:, 0], in_=yj.ap()[0])
        nc.sync.dma_start(out=yr[:, 1], in_=yj.ap()[1])
        nc.sync.dma_start(out=yr[:, 2], in_=yj.ap()[2])
        nc.sync.dma_start(out=yr[:, 3], in_=yj.ap()[3])
        nc.sync.dma_start(out=xr[:, 1], in_=xj.ap()[1])
        nc.sync.dma_start(out=wo_r, in_=wo.ap())
        nc.scalar.dma_start(out=eb, in_=ebias.ap())
        nc.scalar.dma_start(out=wv_r, in_=wv.ap())
        nc.scalar.dma_start(out=xr[:, 2], in_=xj.ap()[2])
        nc.scalar.dma_start(out=xr[:, 3], in_=xj.ap()[3])
        nc.gpsimd.dma_start(out=wq_r, in_=wq.ap())
        nc.gpsimd.dma_start(out=xr[:, 0], in_=xj.ap()[0])

        # ---- PE p-state warmup: dummy matmuls with no input dependency ----
        # The PE clock governor needs ~6us of sustained matmul activity to
        # reach peak; these run during the initial DMA wait.
        warm = res.tile([128, JBLK], BF16, tag='warm')
        nc.vector.memset(warm, 0.125)
        ones_bc = res.tile([128, DH], BF16, tag='onesbc')
        nc.vector.memset(ones_bc, 1.0)
        for wi in range(40):
            wps = ps_g.tile([128, JBLK], F32, tag='g', name=f'warm{wi}')
            nc.tensor.matmul(wps, warm[:, 0:128], warm,
                             start=True, stop=True)
        ones4 = res.tile([128, HPC, 1], F32, tag='ones4')
        nc.vector.memset(ones4, 1.0)

        # ---- resident activations ----
        QT = [res.tile([128, S], BF16, tag=f'qt{p}', name=f'qt{p}') for p in range(2)]
        KTs = [res.tile([128, S], BF16, tag=f'kt{p}', name=f'kt{p}') for p in range(2)]
        v_sb = [res.tile([128, HPC, DH + 1], BF16, tag=f'v{i}', name=f'v{i}')
                for i in range(SK)]

        # ---- projection groups (8 matmuls + eviction) ----
        gid = [0]

        def qk_group(which, p, j4):
            w_r = wq_r if which == 'q' else wk_r
            src = xts if which == 'q' else yts
            dest = QT[p] if which == 'q' else KTs[p]
            js = slice(j4 * JBLK, (j4 + 1) * JBLK)
            cs = slice(p * 128, (p + 1) * 128)
            gid[0] += 1
            gname = f'g{gid[0]}'
            box = {}
            items = []
            for k in range(KT_H):
                def mm(k=k):
                    if k == 0:
                        box['ps'] = ps_g.tile([128, JBLK], F32, tag='g',
                                              name=gname)
                    nc.tensor.matmul(box['ps'], w_r[:, k, cs], src[k][j4],
                                     start=(k == 0), stop=(k == KT_H - 1))
                items.append(mm)

            def fin():
                nc.vector.tensor_copy(dest[:, js], box['ps'])
            items.append(fin)
            return items

        def v_group(sub):
            j4, m = sub // 4, sub % 4
            ms = slice(m * 128, (m + 1) * 128)
            gid[0] += 1
            gname = f'g{gid[0]}'
            box = {}
            items = []
            for k in range(KT_H):
                def mm(k=k):
                    if k == 0:
                        box['ps'] = ps_g.tile([128, JBLK], F32, tag='g',
                                              name=gname)
                    nc.tensor.matmul(box['ps'][:, 0:C], yts[k][j4][:, ms],
                                     wv_r[:, k, :],
                                     start=(k == 0), stop=(k == KT_H - 1))
                items.append(mm)

            def fin():
                nc.vector.tensor_scalar_mul(
                    v_sb[sub][:, :, 0:DH],
                    box['ps'][:, 0:C].rearrange('p (h c) -> p h c', h=HPC),
                    eb[:, sub:sub + 1])
                nc.gpsimd.tensor_scalar_mul(v_sb[sub][:, :, DH:DH + 1], ones4,
                                            eb[:, sub:sub + 1])
            items.append(fin)
            return items

        # prefix: only what the first QK pair needs (V0/V1 go to the weave
        # so the first exp isn't gated on wv/V-compute)
        for grp in (qk_group('k', 0, 0), qk_group('q', 0, 0)):
            for it in grp:
                it()

        # deadline-sorted weave of all remaining projection groups.
        # Deadlines respect BOTH the consumer (a group fully drains when
        # overdue, and must drain at a step before its consumer's QK
        # emission) and the estimated DMA arrival of its source chunk, so
        # a not-yet-ready group never head-of-line-blocks ready work.
        entries = []
        entries.append((0, v_group(0)))
        entries.append((0, v_group(1)))
        entries.append((0, v_group(2)))
        entries.append((0, v_group(3)))
        entries.append((1, qk_group('k', 0, 1)))
        for sub, dl in ((4, 1), (5, 2), (6, 3), (7, 3)):
            entries.append((dl, v_group(sub)))
        entries.append((3, qk_group('k', 1, 0)))
        entries.append((4, qk_group('q', 1, 0)))
        entries.append((4, qk_group('k', 0, 2)))
        for sub, dl in ((8, 5), (9, 5), (10, 6), (11, 6)):
            entries.append((dl, v_group(sub)))
        entries.append((7, qk_group('k', 0, 3)))
        for sub, dl in ((12, 7), (13, 8), (14, 8), (15, 9)):
            entries.append((dl, v_group(sub)))
        for j4 in range(1, NJ):
            entries.append((8 + j4, qk_group('k', 1, j4)))
        for (Jq, p), dl in (((1, 0), 14), ((1, 1), 18), ((2, 0), 40),
                            ((2, 1), 48), ((3, 0), 56), ((3, 1), 64)):
            entries.append((dl, qk_group('q', p, Jq)))
        entries.sort(key=lambda e: e[0])
        weave = [[dl, items, 0] for dl, items in entries]  # [deadline, items, cursor]
        wpos = [0]

        def weave_left():
            return wpos[0] < len(weave)

        def emit_overdue(g):
            n = 0
            while weave_left() and weave[wpos[0]][0] <= g:
                ent = weave[wpos[0]]
                while ent[2] < len(ent[1]):
                    ent[1][ent[2]]()
                    ent[2] += 1
                    n += 1
                wpos[0] += 1
            return n

        def emit_greedy(k):
            n = 0
            while n < k and weave_left():
                ent = weave[wpos[0]]
                ent[1][ent[2]]()
                ent[2] += 1
                n += 1
                if ent[2] == len(ent[1]):
                    wpos[0] += 1
            return n

        # ---- attention + output projection ----
        pend = []

        def out_groups(J, ctx_tiles):
            groups = []
            for m in range(4):
                for n in range(2):
                    def grp(m=m, n=n):
                        ms = slice(m * 128, (m + 1) * 128)
                        ns = slice(n * JBLK, (n + 1) * JBLK)
                        pso = ps_g.tile([128, JBLK], F32, tag='g',
                                        name=f'o{J}_{m}_{n}')
                        for p in range(2):
                            nc.tensor.matmul(pso, ctx_tiles[p][:, ms],
                                             wo_r[:, p, ns],
                                             start=(p == 0), stop=(p == 1))
                        ob = outpool.tile([128, JBLK], F32, tag='ob')
                        nc.vector.tensor_copy(ob, pso)
                        nc.sync.dma_start(
                            out=out.ap()[J * JBLK + m * 128:
                                         J * JBLK + (m + 1) * 128, ns],
                            in_=ob)
                    groups.append(grp)
            return groups

        pairs = [(J, p) for J in range(NJ) for p in range(2)]
        psl_q = []

        def emit_qk(pidx, kk):
            if pidx >= len(pairs):
                return
            J, p = pairs[pidx]
            js = slice(J * JBLK, (J + 1) * JBLK)
            kks = slice(kk * 128, (kk + 1) * 128)
            psl = ps_qk.tile([128, 2 * JBLK], F32, tag='qk',
                             name=f'psl{pidx}_{kk}')
            nc.tensor.matmul(psl[:, 0:JBLK],
                             KTs[p][0:64, kks], QT[p][0:64, js],
                             start=True, stop=True, tile_position=(0, 0))
            nc.tensor.matmul(psl[:, JBLK:2 * JBLK],
                             KTs[p][64:128, kks], QT[p][64:128, js],
                             start=True, stop=True, tile_position=(64, 0))
            psl_q.append(psl)

        emit_qk(0, 0)
        emit_qk(0, 1)
        for J in range(NJ):
            js = slice(J * JBLK, (J + 1) * JBLK)
            ctx_tiles = []
            for p in range(2):
                pidx = J * 2 + p
                pv0 = ps_acc.tile([128, JBLK], F32, tag='acc')
                pv1 = ps_acc.tile([128, JBLK], F32, tag='acc')
                for kk in range(SK):
                    g = pidx * SK + kk
                    # QK two steps ahead, crossing pair boundaries
                    if kk + 2 < SK:
                        emit_qk(pidx, kk + 2)
                    else:
                        emit_qk(pidx + 1, kk + 2 - SK)
                    psl = psl_q.pop(0)
                    emitted = emit_overdue(g)
                    if not emitted:
                        if pend and (kk % 2 == 0 or not weave_left()):
                            pend.pop(0)()
                        else:
                            emit_greedy(2)
                    ex = expool.tile([128, 2 * JBLK], BF16, tag='ex')
                    nc.scalar.activation(ex, psl,
                                         mybir.ActivationFunctionType.Exp)
                    if _DEBUG and pidx == 0 and kk == 0:
                        de = outpool.tile([128, 2 * JBLK], F32, tag='de',
                                          bufs=1)
                        nc.vector.tensor_copy(de, ex)
                        nc.sync.dma_start(out=dbg['d_ex'].ap(), in_=de)
                    for hh, pv in enumerate((pv0, pv1)):
                        hcol = 2 * p + hh
                        nc.tensor.matmul(
                            pv[0:DH + 1, :],
                            v_sb[kk][:, hcol, :],
                            ex[:, hh * JBLK:(hh + 1) * JBLK],
                            start=(kk == 0), stop=(kk == SK - 1))
                # normalize: ctxT[d, q] * (1/denom[q])
                ct = ctxpool.tile([128, JBLK], BF16, tag=f'ctx{p}')
                if J == NJ - 1 and p == 1:
                    # tail-critical pair: broadcast 1/denom across partitions
                    # with a K=1 matmul (tensor is idle here) instead of the
                    # slow DMA-hop + gpsimd chain; warm mms keep the PE clock
                    # up through the vector chain.
                    for wi in range(2):
                        wps = ps_g.tile([128, JBLK], F32, tag='g',
                                        name=f'twm{wi}')
                        nc.tensor.matmul(wps, warm[:, 0:128], warm,
                                         start=True, stop=True)
                    for hh, pv in enumerate((pv0, pv1)):
                        rawct = small.tile([128, JBLK], F32, tag='rawct')
                        nc.vector.tensor_copy(rawct[0:DH + 1, :],
                                              pv[0:DH + 1, :])
                        rec = small.tile([128, JBLK], F32, tag='rec')
                        nc.vector.reciprocal_approx_fast(rec[0:DH + 1, :],
                                                         rawct[0:DH + 1, :])
                        recb = small.tile([128, JBLK], BF16, tag='recb')
                        nc.vector.tensor_copy(recb[DH:DH + 1, :],
                                              rec[DH:DH + 1, :])
                        bcps = ps_g.tile([128, JBLK], F32, tag='g',
                                         name=f'bc{hh}')
                        nc.tensor.matmul(bcps[0:DH, :], ones_bc[DH:DH + 1, :],
                                         recb[DH:DH + 1, :],
                                         start=True, stop=True,
                                         tile_position=(64, 0))
                        if hh == 0:
                            nc.vector.tensor_mul(ct[0:DH, :], rawct[0:DH, :],
                                                 bcps[0:DH, :])
                        else:
                            tmp = small.tile([128, JBLK], BF16, tag='tmp')
                            nc.vector.tensor_mul(tmp[0:DH, :], rawct[0:DH, :],
                                                 bcps[0:DH, :])
                            nc.sync.dma_start(out=ct[DH:128, :],
                                              in_=tmp[0:DH, :])
                else:
                    stage = []
                    for hh, pv in enumerate((pv0, pv1)):
                        rawct = small.tile([128, JBLK], F32, tag='rawct')
                        nc.vector.tensor_copy(rawct[0:DH + 1, :],
                                              pv[0:DH + 1, :])
                        rec = small.tile([128, JBLK], F32, tag='rec')
                        nc.vector.reciprocal_approx_fast(rec[0:DH + 1, :],
                                                         rawct[0:DH + 1, :])
                        bcs = small.tile([128, JBLK], F32, tag='bcs')
                        nc.sync.dma_start(out=bcs[0:1, :], in_=rec[DH:DH + 1, :])
                        bc = small.tile([128, JBLK], F32, tag='bc')
                        nc.gpsimd.partition_broadcast(bc[0:DH, :], bcs[0:1, :])
                        stage.append((rawct, bc))
                    for hh, (rawct, bc) in enumerate(stage):
                        if hh == 0:
                            nc.vector.tensor_mul(ct[0:DH, :], rawct[0:DH, :],
                                                 bc[0:DH, :])
                        else:
                            tmp = small.tile([128, JBLK], BF16, tag='tmp')
                            nc.vector.tensor_mul(tmp[0:DH, :], rawct[0:DH, :],
                                                 bc[0:DH, :])
                            nc.sync.dma_start(out=ct[DH:128, :],
                                              in_=tmp[0:DH, :])
                if _DEBUG and pidx == 0:
                    dc = outpool.tile([128, JBLK], F32, tag='dc', bufs=1)
                    nc.vector.tensor_copy(dc, ct)
                    nc.sync.dma_start(out=dbg['d_ctx'].ap(), in_=dc)
                    dr = outpool.tile([128, JBLK], F32, tag='dr', bufs=1)
                    nc.vector.tensor_copy(dr[0:DH + 1, :],
                                          stage[0][0][0:DH + 1, :])
                    nc.sync.dma_start(out=dbg['d_raw'].ap(), in_=dr)
                ctx_tiles.append(ct)
            for grp in pend:       # drain any leftovers before replacing
                grp()
            if J < NJ - 1:
                pend = out_groups(J, ctx_tiles)
            else:
                last_ctx = ctx_tiles
        while weave_left():
            emit_greedy(9)
        # ---- tail: final q-block's output projection ----
        # Alternate psum across two pools (4 banks in flight) and evictions
        # across vector/scalar (scalar is idle after the last exp) so the
        # tail is paced by the matmul stream + the final 2MB HBM write.
        # Warm matmuls keep the PE clock up through the normalize chain and
        # the ct partition-shift DMA wait.
        for wi in range(10):
            wps = ps_acc.tile([128, JBLK], F32, tag='acc', name=f'fwm{wi}')
            nc.tensor.matmul(wps, warm[:, 0:128], warm, start=True, stop=True)
        for gi in range(8):
            m, n = gi // 2, gi % 2
            ms = slice(m * 128, (m + 1) * 128)
            ns = slice(n * JBLK, (n + 1) * JBLK)
            pool, tag = (ps_g, 'g') if gi % 2 == 0 else (ps_acc, 'acc')
            pso = pool.tile([128, JBLK], F32, tag=tag, name=f'fo{gi}')
            for p in range(2):
                nc.tensor.matmul(pso, last_ctx[p][:, ms], wo_r[:, p, ns],
                                 start=(p == 0), stop=(p == 1))
            ob = outpool.tile([128, JBLK], F32, tag='ob')
            if gi % 2 == 0:
                nc.vector.tensor_copy(ob, pso)
            else:
                nc.scalar.copy(ob, pso)
            nc.sync.dma_start(
                out=out.ap()[(NJ - 1) * JBLK + m * 128:
                             (NJ - 1) * JBLK + (m + 1) * 128, ns],
                in_=ob)
        if _DEBUG:
            for nm, src, w in [('d_kt', KTs[0], S), ('d_qt', QT[0], S),
                               ('d_v', v_sb[0].rearrange('p h c -> p (h c)'),
                                HPC * (DH + 1)),
                               ('d_wo', wo_r.rearrange('p t n -> p (t n)'),
                                2 * H)]:
                dd = res.tile([128, w], F32, tag=f'dd{nm}', name=f'dd{nm}')
                nc.vector.tensor_copy(dd, src)
                nc.sync.dma_start(out=dbg[nm].ap(), in_=dd)

    nc.compile()
    return nc


def _get_nc():
    if 'nc' not in _CACHE:
        _CACHE['nc'] = _build()
    return _CACHE['nc']


def _tile_act(aT):
    """[H, S] -> [NJ, 128, KT_H, JBLK]: j-major, partition-major chunks."""
    t = aT.reshape(KT_H, 128, NJ, JBLK)
    return np.ascontiguousarray(t.transpose(2, 1, 0, 3))


def _tile_w(w):
    """[H, C] -> [128, KT_H, C]."""
    return np.ascontiguousarray(w.reshape(KT_H, 128, C).transpose(1, 0, 2))


def shard_inputs(x, y, bias, wq, wk, wv, wo):
    """Build the 8 per-core input maps from full inputs."""
    scale = (H // NH) ** -0.5
    wqs = (wq * scale).astype(np.float32)
    bf = ml_dtypes.bfloat16
    in_maps = []
    for c in range(NCORES):
        b = c // (NCORES // B)
        g = c % (NCORES // B)
        cols = slice(g * C, (g + 1) * C)
        eb = np.exp(bias[b, 0, 0, :].astype(np.float64)).astype(np.float32)
        in_maps.append({
            'xj': _tile_act(x[b].T.astype(bf)),
            'yj': _tile_act(y[b].T.astype(bf)),
            'wq': _tile_w(wqs[:, cols].astype(bf)),
            'wk': _tile_w(wk[:, cols].astype(bf)),
            'wv': _tile_w(wv[:, cols].astype(bf)),
            'wo': np.ascontiguousarray(
                wo[cols, :].astype(bf).reshape(2, 128, H).transpose(1, 0, 2)),
            'ebias': np.ascontiguousarray(eb.reshape(SK, 128).T),
        })
    return in_maps


def kernel(x, y, bias, wq, wk, wv, wo, _trace=False):
    x, y, bias = np.asarray(x), np.asarray(y), np.asarray(bias)
    wq, wk, wv, wo = (np.asarray(t) for t in (wq, wk, wv, wo))
    nc = _get_nc()
    in_maps = shard_inputs(x, y, bias, wq, wk, wv, wo)
    kw = {}
    if _trace:
        kw = dict(trace=True, stitch_traces=False)
    res = bass_utils.run_bass_kernel_spmd(nc, in_maps, core_ids=list(range(NCORES)), **kw)
    full = np.zeros((B, S, H), dtype=np.float64)
    for c in range(NCORES):
        full[c // (NCORES // B)] += res.results[c]['out'].astype(np.float64)
    if _trace:
        _CACHE['last_results'] = res
    return full.astype(np.float32)



# revision 41
# speedup vs baseline: 1.0020x; 1.0020x over previous
"""Multi-head attention (B=2, S=2048, H=1024, 16 heads) on 8 TRN2 NeuronCores.

Sharding: data parallel on batch (2) x tensor parallel on heads (4 heads/core,
Megatron column-split qkv, row-split wo). Host pre-transposes and pre-TILES
x/y/weights into the exact SBUF layouts (so every input DMA is a handful of
fully-contiguous 4-8KB-per-partition transfers; the HWDGE rings are
descriptor-rate-bound), pre-scales wq by dh^-0.5, and sum-reduces the 4
partial outputs per batch element.

Per-core kernel (v3):
  Steady state is tensor-issue-bound at ~1.19us/step (QK pair 0.31 + PV 0.43
  + woven projections 0.44); the exp chain floor is ~1.0us/step (back-to-back
  ACTIVATEs overlap their pipeline fill/drain). The first ~16 steps are
  DMA-arrival + V/K-projection-crunch bound: both HWDGE rings lead with y
  (K-proj and V-proj both consume it), j0 split in halves so the first
  projection group starts on the first half-MB; x-j123/wo ride last. Weave
  deadlines are arrival-aware so a not-yet-ready group never head-of-line
  blocks ready work, and Q-projections for later q-blocks are pulled into
  the post-crunch trough so late steps run ACT-bound. Warmup matmuls ramp
  the PE clock governor (~6us of sustained activity to reach peak; it
  drops back during idle gaps).

  Attention per 512-wide q-block and head pair: row-tiled (2-head packed)
  QK^T -> logitsT psum [128,1024] -> one ACT exp per pair (psum->sbuf bf16)
  -> PV matmul with fused denominator row (fp32 accumulate) -> fast
  reciprocal + gpsimd partition_broadcast normalize -> pair-stacked bf16
  output projection, deferred one block for overlap. The final pair instead
  normalizes via a K=1 broadcast matmul (tensor is idle at the tail) and the
  final q-block's output projection alternates psum pools and vector/scalar
  evictions with immediate per-chunk DMA. Output stays f32 (bf16 DRAM
  outputs do not land on HW). fp8 was tried and rejected: e4m3 DoubleRow
  halves instructions but NOT stream time (no throughput win), and q/k-proj
  quantization alone costs 3.2e-2 rel err vs the 2e-2 gate.
"""
import sys
sys.path.insert(0, '/opt/trn_rl_repo')
from contextlib import ExitStack

import numpy as np
import ml_dtypes

import concourse.bacc as bacc
import concourse.tile as tile
from concourse import mybir
from concourse import bass_utils

B, S, H, NH = 2, 2048, 1024, 16
DH = H // NH            # 64
NCORES = 8
HPC = NH // (NCORES // B)   # 4 heads per core
C = HPC * DH            # 256 projected cols per core
KT_H = H // 128         # 8 contraction tiles over H
SK = S // 128           # 16 s-subtiles
JBLK = 512
NJ = S // JBLK          # 4 q-blocks
F32 = mybir.dt.float32
BF16 = mybir.dt.bfloat16

_CACHE = {}
_DEBUG = False


def _build():
    nc = bacc.Bacc('TRN2', debug=False, num_devices=NCORES)
    # All activations/weights arrive HOST-PRE-TILED into the SBUF layout so
    # every chunk DMA is fully contiguous per partition (8KB lines): the
    # HWDGE rings are descriptor-rate-bound, so line size sets arrival BW.
    xj = nc.dram_tensor('xj', [NJ, 128, KT_H, JBLK], BF16, kind='ExternalInput')
    yj = nc.dram_tensor('yj', [NJ, 128, KT_H, JBLK], BF16, kind='ExternalInput')
    wq = nc.dram_tensor('wq', [128, KT_H, C], BF16, kind='ExternalInput')
    wk = nc.dram_tensor('wk', [128, KT_H, C], BF16, kind='ExternalInput')
    wv = nc.dram_tensor('wv', [128, KT_H, C], BF16, kind='ExternalInput')
    wo = nc.dram_tensor('wo', [128, 2, H], BF16, kind='ExternalInput')
    ebias = nc.dram_tensor('ebias', [128, SK], F32, kind='ExternalInput')
    out = nc.dram_tensor('out', [S, H], F32, kind='ExternalOutput')
    dbg = {}
    if _DEBUG:
        for nm, w in [('d_kt', S), ('d_qt', S), ('d_ex', 2 * JBLK),
                      ('d_v', HPC * (DH + 1)), ('d_wo', 2 * H),
                      ('d_ctx', JBLK), ('d_raw', JBLK)]:
            dbg[nm] = nc.dram_tensor(nm, [128, w], F32, kind='ExternalOutput')

    with tile.TileContext(nc) as tc, ExitStack() as ctx:
        res = ctx.enter_context(tc.tile_pool(name='res', bufs=1))
        expool = ctx.enter_context(tc.tile_pool(name='expool', bufs=4))
        ctxpool = ctx.enter_context(tc.tile_pool(name='ctxpool', bufs=2))
        small = ctx.enter_context(tc.tile_pool(name='small', bufs=3))
        outpool = ctx.enter_context(tc.tile_pool(name='outpool', bufs=4))
        ps_qk = ctx.enter_context(tc.tile_pool(name='ps_qk', bufs=2, space='PSUM'))
        ps_acc = ctx.enter_context(tc.tile_pool(name='ps_acc', bufs=2, space='PSUM'))
        ps_g = ctx.enter_context(tc.tile_pool(name='ps_g', bufs=2, space='PSUM'))

        # ---- input DMAs: few, large, contiguous, split across both rings ----
        # HWDGE rings execute FIFO per issuing engine, so arrival order ==
        # issue order per ring; the two rings share the ~358GB/s HBM pipe.
        # K/V-path (wk, y, wv) rides sync; Q-path (wq, x) rides scalar.
        eb = res.tile([128, SK], F32, tag='eb')
        wk_r = res.tile([128, KT_H, C], BF16, tag='wk')
        wv_r = res.tile([128, KT_H, C], BF16, tag='wv')
        wq_r = res.tile([128, KT_H, C], BF16, tag='wq')
        wo_r = res.tile([128, 2, H], BF16, tag='wo')
        xr = res.tile([128, NJ, KT_H, JBLK], BF16, tag='xr')
        yr = res.tile([128, NJ, KT_H, JBLK], BF16, tag='yr')
        xts = [[xr[:, j, k] for j in range(NJ)] for k in range(KT_H)]
        yts = [[yr[:, j, k] for j in range(NJ)] for k in range(KT_H)]

        # Each dma_start costs ~2us of serial ring time on top of transfer
        # (completion receipt), so the critical path uses FEW, LARGE DMAs
        # spread over THREE rings (sync + scalar HWDGE, gpsimd SWDGE). y
        # gates the whole first q-block (K-proj and V-proj both consume
        # it); x-j123/wo aren't touched until step ~26+ and ride last.
        # Ring constraints: a ring's queue is FIFO, and an engine's OTHER
        # work queues behind its DMA transfers. So the scalar ring carries
        # only tiny DMAs that finish before the first exp; the gpsimd ring
        # only what finishes before its first in-loop op (V-denominator
        # muls); everything else rides sync in consumption order.
        nc.sync.dma_start(out=wk_r, in_=wk.ap())
        nc.sync.dma_start(out=yr[:, 0], in_=yj.ap()[0])
        nc.sync.dma_start(out=yr[:, 1], in_=yj.ap()[1])
        nc.sync.dma_start(out=xr[:, 1], in_=xj.ap()[1])
        nc.sync.dma_start(out=wo_r, in_=wo.ap())
        nc.scalar.dma_start(out=eb, in_=ebias.ap())
        nc.scalar.dma_start(out=wv_r, in_=wv.ap())
        nc.scalar.dma_start(out=xr[:, 2], in_=xj.ap()[2])
        nc.scalar.dma_start(out=xr[:, 3], in_=xj.ap()[3])
        nc.gpsimd.dma_start(out=wq_r, in_=wq.ap())
        nc.gpsimd.dma_start(out=xr[:, 0], in_=xj.ap()[0])
        nc.gpsimd.dma_start(out=yr[:, 2], in_=yj.ap()[2])
        nc.gpsimd.dma_start(out=yr[:, 3], in_=yj.ap()[3])

        # ---- PE p-state warmup: dummy matmuls with no input dependency ----
        # The PE clock governor needs ~6us of sustained matmul activity to
        # reach peak; these run during the initial DMA wait.
        warm = res.tile([128, JBLK], BF16, tag='warm')
        nc.vector.memset(warm, 0.125)
        ones_bc = res.tile([128, DH], BF16, tag='onesbc')
        nc.vector.memset(ones_bc, 1.0)
        for wi in range(40):
            wps = ps_g.tile([128, JBLK], F32, tag='g', name=f'warm{wi}')
            nc.tensor.matmul(wps, warm[:, 0:128], warm,
                             start=True, stop=True)
        ones4 = res.tile([128, HPC, 1], F32, tag='ones4')
        nc.vector.memset(ones4, 1.0)

        # ---- resident activations ----
        QT = [res.tile([128, S], BF16, tag=f'qt{p}', name=f'qt{p}') for p in range(2)]
        KTs = [res.tile([128, S], BF16, tag=f'kt{p}', name=f'kt{p}') for p in range(2)]
        v_sb = [res.tile([128, HPC, DH + 1], BF16, tag=f'v{i}', name=f'v{i}')
                for i in range(SK)]

        # ---- projection groups (8 matmuls + eviction) ----
        gid = [0]

        def qk_group(which, p, j4):
            w_r = wq_r if which == 'q' else wk_r
            src = xts if which == 'q' else yts
            dest = QT[p] if which == 'q' else KTs[p]
            js = slice(j4 * JBLK, (j4 + 1) * JBLK)
            cs = slice(p * 128, (p + 1) * 128)
            gid[0] += 1
            gname = f'g{gid[0]}'
            box = {}
            items = []
            for k in range(KT_H):
                def mm(k=k):
                    if k == 0:
                        box['ps'] = ps_g.tile([128, JBLK], F32, tag='g',
                                              name=gname)
                    nc.tensor.matmul(box['ps'], w_r[:, k, cs], src[k][j4],
                                     start=(k == 0), stop=(k == KT_H - 1))
                items.append(mm)

            def fin():
                nc.vector.tensor_copy(dest[:, js], box['ps'])
            items.append(fin)
            return items

        def v_group(sub):
            j4, m = sub // 4, sub % 4
            ms = slice(m * 128, (m + 1) * 128)
            gid[0] += 1
            gname = f'g{gid[0]}'
            box = {}
            items = []
            for k in range(KT_H):
                def mm(k=k):
                    if k == 0:
                        box['ps'] = ps_g.tile([128, JBLK], F32, tag='g',
                                              name=gname)
                    nc.tensor.matmul(box['ps'][:, 0:C], yts[k][j4][:, ms],
                                     wv_r[:, k, :],
                                     start=(k == 0), stop=(k == KT_H - 1))
                items.append(mm)

            def fin():
                nc.vector.tensor_scalar_mul(
                    v_sb[sub][:, :, 0:DH],
                    box['ps'][:, 0:C].rearrange('p (h c) -> p h c', h=HPC),
                    eb[:, sub:sub + 1])
                # denominator row on vector too: keeps the gpsimd queue free
                # to serve as a third DMA ring during the crunch
                nc.vector.tensor_scalar_mul(v_sb[sub][:, :, DH:DH + 1], ones4,
                                            eb[:, sub:sub + 1])
            items.append(fin)
            return items

        # prefix: only what the first QK pair needs (V0/V1 go to the weave
        # so the first exp isn't gated on wv/V-compute)
        for grp in (qk_group('k', 0, 0), qk_group('q', 0, 0)):
            for it in grp:
                it()

        # deadline-sorted weave of all remaining projection groups.
        # Deadlines respect BOTH the consumer (a group fully drains when
        # overdue, and must drain at a step before its consumer's QK
        # emission) and the estimated DMA arrival of its source chunk, so
        # a not-yet-ready group never head-of-line-blocks ready work.
        entries = []
        entries.append((0, v_group(0)))
        entries.append((0, v_group(1)))
        entries.append((0, v_group(2)))
        entries.append((0, v_group(3)))
        entries.append((1, qk_group('k', 0, 1)))
        for sub, dl in ((4, 1), (5, 2), (6, 3), (7, 3)):
            entries.append((dl, v_group(sub)))
        entries.append((3, qk_group('k', 1, 0)))
        entries.append((4, qk_group('q', 1, 0)))
        entries.append((4, qk_group('k', 0, 2)))
        for sub, dl in ((8, 5), (9, 5), (10, 6), (11, 6)):
            entries.append((dl, v_group(sub)))
        entries.append((7, qk_group('k', 0, 3)))
        for sub, dl in ((12, 7), (13, 8), (14, 8), (15, 9)):
            entries.append((dl, v_group(sub)))
        for j4 in range(1, NJ):
            entries.append((8 + j4, qk_group('k', 1, j4)))
        for (Jq, p), dl in (((1, 0), 14), ((1, 1), 18), ((2, 0), 40),
                            ((2, 1), 48), ((3, 0), 56), ((3, 1), 64)):
            entries.append((dl, qk_group('q', p, Jq)))
        entries.sort(key=lambda e: e[0])
        weave = [[dl, items, 0] for dl, items in entries]  # [deadline, items, cursor]
        wpos = [0]

        def weave_left():
            return wpos[0] < len(weave)

        def emit_overdue(g):
            n = 0
            while weave_left() and weave[wpos[0]][0] <= g:
                ent = weave[wpos[0]]
                while ent[2] < len(ent[1]):
                    ent[1][ent[2]]()
                    ent[2] += 1
                    n += 1
                wpos[0] += 1
            return n

        def emit_greedy(k):
            n = 0
            while n < k and weave_left():
                ent = weave[wpos[0]]
                ent[1][ent[2]]()
                ent[2] += 1
                n += 1
                if ent[2] == len(ent[1]):
                    wpos[0] += 1
            return n

        # ---- attention + output projection ----
        pend = []

        def out_groups(J, ctx_tiles):
            groups = []
            for m in range(4):
                for n in range(2):
                    def grp(m=m, n=n):
                        ms = slice(m * 128, (m + 1) * 128)
                        ns = slice(n * JBLK, (n + 1) * JBLK)
                        pso = ps_g.tile([128, JBLK], F32, tag='g',
                                        name=f'o{J}_{m}_{n}')
                        for p in range(2):
                            nc.tensor.matmul(pso, ctx_tiles[p][:, ms],
                                             wo_r[:, p, ns],
                                             start=(p == 0), stop=(p == 1))
                        ob = outpool.tile([128, JBLK], F32, tag='ob')
                        nc.vector.tensor_copy(ob, pso)
                        nc.sync.dma_start(
                            out=out.ap()[J * JBLK + m * 128:
                                         J * JBLK + (m + 1) * 128, ns],
                            in_=ob)
                    groups.append(grp)
            return groups

        pairs = [(J, p) for J in range(NJ) for p in range(2)]
        psl_q = []

        def emit_qk(pidx, kk):
            if pidx >= len(pairs):
                return
            J, p = pairs[pidx]
            js = slice(J * JBLK, (J + 1) * JBLK)
            kks = slice(kk * 128, (kk + 1) * 128)
            psl = ps_qk.tile([128, 2 * JBLK], F32, tag='qk',
                             name=f'psl{pidx}_{kk}')
            nc.tensor.matmul(psl[:, 0:JBLK],
                             KTs[p][0:64, kks], QT[p][0:64, js],
                             start=True, stop=True, tile_position=(0, 0))
            nc.tensor.matmul(psl[:, JBLK:2 * JBLK],
                             KTs[p][64:128, kks], QT[p][64:128, js],
                             start=True, stop=True, tile_position=(64, 0))
            psl_q.append(psl)

        emit_qk(0, 0)
        emit_qk(0, 1)
        for J in range(NJ):
            js = slice(J * JBLK, (J + 1) * JBLK)
            ctx_tiles = []
            for p in range(2):
                pidx = J * 2 + p
                pv0 = ps_acc.tile([128, JBLK], F32, tag='acc')
                pv1 = ps_acc.tile([128, JBLK], F32, tag='acc')
                for kk in range(SK):
                    g = pidx * SK + kk
                    # QK two steps ahead, crossing pair boundaries
                    if kk + 2 < SK:
                        emit_qk(pidx, kk + 2)
                    else:
                        emit_qk(pidx + 1, kk + 2 - SK)
                    psl = psl_q.pop(0)
                    emitted = emit_overdue(g)
                    if not emitted:
                        if pend and (kk % 2 == 0 or not weave_left()):
                            pend.pop(0)()
                        else:
                            emit_greedy(2)
                    ex = expool.tile([128, 2 * JBLK], BF16, tag='ex')
                    nc.scalar.activation(ex, psl,
                                         mybir.ActivationFunctionType.Exp)
                    if _DEBUG and pidx == 0 and kk == 0:
                        de = outpool.tile([128, 2 * JBLK], F32, tag='de',
                                          bufs=1)
                        nc.vector.tensor_copy(de, ex)
                        nc.sync.dma_start(out=dbg['d_ex'].ap(), in_=de)
                    for hh, pv in enumerate((pv0, pv1)):
                        hcol = 2 * p + hh
                        nc.tensor.matmul(
                            pv[0:DH + 1, :],
                            v_sb[kk][:, hcol, :],
                            ex[:, hh * JBLK:(hh + 1) * JBLK],
                            start=(kk == 0), stop=(kk == SK - 1))
                # normalize: ctxT[d, q] * (1/denom[q])
                ct = ctxpool.tile([128, JBLK], BF16, tag=f'ctx{p}')
                if J == NJ - 1 and p == 1:
                    # tail-critical pair: broadcast 1/denom across partitions
                    # with a K=1 matmul (tensor is idle here) instead of the
                    # slow DMA-hop + gpsimd chain; warm mms keep the PE clock
                    # up through the vector chain.
                    for wi in range(2):
                        wps = ps_g.tile([128, JBLK], F32, tag='g',
                                        name=f'twm{wi}')
                        nc.tensor.matmul(wps, warm[:, 0:128], warm,
                                         start=True, stop=True)
                    for hh, pv in enumerate((pv0, pv1)):
                        rawct = small.tile([128, JBLK], F32, tag='rawct')
                        nc.vector.tensor_copy(rawct[0:DH + 1, :],
                                              pv[0:DH + 1, :])
                        rec = small.tile([128, JBLK], F32, tag='rec')
                        nc.vector.reciprocal_approx_fast(rec[0:DH + 1, :],
                                                         rawct[0:DH + 1, :])
                        recb = small.tile([128, JBLK], BF16, tag='recb')
                        nc.vector.tensor_copy(recb[DH:DH + 1, :],
                                              rec[DH:DH + 1, :])
                        bcps = ps_g.tile([128, JBLK], F32, tag='g',
                                         name=f'bc{hh}')
                        nc.tensor.matmul(bcps[0:DH, :], ones_bc[DH:DH + 1, :],
                                         recb[DH:DH + 1, :],
                                         start=True, stop=True,
                                         tile_position=(64, 0))
                        if hh == 0:
                            nc.vector.tensor_mul(ct[0:DH, :], rawct[0:DH, :],
                                                 bcps[0:DH, :])
                        else:
                            tmp = small.tile([128, JBLK], BF16, tag='tmp')
                            nc.vector.tensor_mul(tmp[0:DH, :], rawct[0:DH, :],
                                                 bcps[0:DH, :])
                            nc.sync.dma_start(out=ct[DH:128, :],
                                              in_=tmp[0:DH, :])
                else:
                    stage = []
                    for hh, pv in enumerate((pv0, pv1)):
                        rawct = small.tile([128, JBLK], F32, tag='rawct')
                        nc.vector.tensor_copy(rawct[0:DH + 1, :],
                                              pv[0:DH + 1, :])
                        rec = small.tile([128, JBLK], F32, tag='rec')
                        nc.vector.reciprocal_approx_fast(rec[0:DH + 1, :],
                                                         rawct[0:DH + 1, :])
                        bcs = small.tile([128, JBLK], F32, tag='bcs')
                        nc.sync.dma_start(out=bcs[0:1, :], in_=rec[DH:DH + 1, :])
                        bc = small.tile([128, JBLK], F32, tag='bc')
                        nc.gpsimd.partition_broadcast(bc[0:DH, :], bcs[0:1, :])
                        stage.append((rawct, bc))
                    for hh, (rawct, bc) in enumerate(stage):
                        if hh == 0:
                            nc.vector.tensor_mul(ct[0:DH, :], rawct[0:DH, :],
                                                 bc[0:DH, :])
                        else:
                            tmp = small.tile([128, JBLK], BF16, tag='tmp')
                            nc.vector.tensor_mul(tmp[0:DH, :], rawct[0:DH, :],
                                                 bc[0:DH, :])
                            nc.sync.dma_start(out=ct[DH:128, :],
                                              in_=tmp[0:DH, :])
                if _DEBUG and pidx == 0:
                    dc = outpool.tile([128, JBLK], F32, tag='dc', bufs=1)
                    nc.vector.tensor_copy(dc, ct)
                    nc.sync.dma_start(out=dbg['d_ctx'].ap(), in_=dc)
                    dr = outpool.tile([128, JBLK], F32, tag='dr', bufs=1)
                    nc.vector.tensor_copy(dr[0:DH + 1, :],
                                          stage[0][0][0:DH + 1, :])
                    nc.sync.dma_start(out=dbg['d_raw'].ap(), in_=dr)
                ctx_tiles.append(ct)
            for grp in pend:       # drain any leftovers before replacing
                grp()
            if J < NJ - 1:
                pend = out_groups(J, ctx_tiles)
            else:
                last_ctx = ctx_tiles
        while weave_left():
            emit_greedy(9)
        # ---- tail: final q-block's output projection ----
        # Alternate psum across two pools (4 banks in flight) and evictions
        # across vector/scalar (scalar is idle after the last exp) so the
        # tail is paced by the matmul stream + the final 2MB HBM write.
        # Warm matmuls keep the PE clock up through the normalize chain and
        # the ct partition-shift DMA wait.
        for wi in range(10):
            wps = ps_acc.tile([128, JBLK], F32, tag='acc', name=f'fwm{wi}')
            nc.tensor.matmul(wps, warm[:, 0:128], warm, start=True, stop=True)
        for gi in range(8):
            m, n = gi // 2, gi % 2
            ms = slice(m * 128, (m + 1) * 128)
            ns = slice(n * JBLK, (n + 1) * JBLK)
            pool, tag = (ps_g, 'g') if gi % 2 == 0 else (ps_acc, 'acc')
            pso = pool.tile([128, JBLK], F32, tag=tag, name=f'fo{gi}')
            for p in range(2):
                nc.tensor.matmul(pso, last_ctx[p][:, ms], wo_r[:, p, ns],
                                 start=(p == 0), stop=(p == 1))
            ob = outpool.tile([128, JBLK], F32, tag='ob')
            if gi % 2 == 0:
                nc.vector.tensor_copy(ob, pso)
            else:
                nc.scalar.copy(ob, pso)
            nc.sync.dma_start(
                out=out.ap()[(NJ - 1) * JBLK + m * 128:
                             (NJ - 1) * JBLK + (m + 1) * 128, ns],
                in_=ob)
        if _DEBUG:
            for nm, src, w in [('d_kt', KTs[0], S), ('d_qt', QT[0], S),
                               ('d_v', v_sb[0].rearrange('p h c -> p (h c)'),
                                HPC * (DH + 1)),
                               ('d_wo', wo_r.rearrange('p t n -> p (t n)'),
                                2 * H)]:
                dd = res.tile([128, w], F32, tag=f'dd{nm}', name=f'dd{nm}')
                nc.vector.tensor_copy(dd, src)
                nc.sync.dma_start(out=dbg[nm].ap(), in_=dd)

    nc.compile()
    return nc


def _get_nc():
    if 'nc' not in _CACHE:
        _CACHE['nc'] = _build()
    return _CACHE['nc']


def _tile_act(aT):
    """[H, S] -> [NJ, 128, KT_H, JBLK]: j-major, partition-major chunks."""
    t = aT.reshape(KT_H, 128, NJ, JBLK)
    return np.ascontiguousarray(t.transpose(2, 1, 0, 3))


def _tile_w(w):
    """[H, C] -> [128, KT_H, C]."""
    return np.ascontiguousarray(w.reshape(KT_H, 128, C).transpose(1, 0, 2))


def shard_inputs(x, y, bias, wq, wk, wv, wo):
    """Build the 8 per-core input maps from full inputs."""
    scale = (H // NH) ** -0.5
    wqs = (wq * scale).astype(np.float32)
    bf = ml_dtypes.bfloat16
    in_maps = []
    for c in range(NCORES):
        b = c // (NCORES // B)
        g = c % (NCORES // B)
        cols = slice(g * C, (g + 1) * C)
        eb = np.exp(bias[b, 0, 0, :].astype(np.float64)).astype(np.float32)
        in_maps.append({
            'xj': _tile_act(x[b].T.astype(bf)),
            'yj': _tile_act(y[b].T.astype(bf)),
            'wq': _tile_w(wqs[:, cols].astype(bf)),
            'wk': _tile_w(wk[:, cols].astype(bf)),
            'wv': _tile_w(wv[:, cols].astype(bf)),
            'wo': np.ascontiguousarray(
                wo[cols, :].astype(bf).reshape(2, 128, H).transpose(1, 0, 2)),
            'ebias': np.ascontiguousarray(eb.reshape(SK, 128).T),
        })
    return in_maps


def kernel(x, y, bias, wq, wk, wv, wo, _trace=False):
    x, y, bias = np.asarray(x), np.asarray(y), np.asarray(bias)
    wq, wk, wv, wo = (np.asarray(t) for t in (wq, wk, wv, wo))
    nc = _get_nc()
    in_maps = shard_inputs(x, y, bias, wq, wk, wv, wo)
    kw = {}
    if _trace:
        kw = dict(trace=True, stitch_traces=False)
    res = bass_utils.run_bass_kernel_spmd(nc, in_maps, core_ids=list(range(NCORES)), **kw)
    full = np.zeros((B, S, H), dtype=np.float64)
    for c in range(NCORES):
        full[c // (NCORES // B)] += res.results[c]['out'].astype(np.float64)
    if _trace:
        _CACHE['last_results'] = res
    return full.astype(np.float32)

